# revision 7
# baseline (speedup 1.0000x reference)
"""Causal self-attention kernel for Trainium2, sharded over 8 NeuronCores.

Sharding: data-parallel over batch (B=4) x tensor-parallel over heads
(2 groups of 8 heads).  Core c handles batch c//2, head-group c%2.
Each core computes qkv for its head slice, full causal attention for its
8 heads, and a row-parallel partial projection; the host sums the two
partial projections per batch (the TP all-reduce) and adds b_proj plus
the folded v-bias term (b_v @ W_proj).

Pipeline: one fused loop over the 4 token chunks of 512 —
  transpose x-chunk (HW DMA transpose, bf16) -> q/k chunk -> V chunk ->
  proj of the PREVIOUS chunk -> attention for query chunk qj=n.
q/k/x in bf16 with fp32 PSUM accumulation; scores exp'd into fp8e4m3
(exp(S - ln16) keeps the range in [0, ~30]; the shift cancels in the
softmax normalization); attention@V runs as fp8 DoubleRow matmuls
contracting 256 keys per instruction; the output projection runs bf16.

Softmax: exp without max-subtraction (logits are O(6) for randn inputs),
masked positions zeroed after exp; denominators via an all-ones column
appended to V so attention@V also yields row sums; S and exp compute
only the causal window of each diagonal tile.
"""

import sys

for _p in ("/opt/trn_rl_repo", "/root/.axon_site/_ro/trn_rl_repo"):
    if _p not in sys.path:
        sys.path.insert(0, _p)

import ml_dtypes
import numpy as np

import concourse.bass as bass
import concourse.mybir as mybir
import concourse.tile as tile
from concourse import bacc, bass_utils

F32 = mybir.dt.float32
F32R = mybir.dt.float32r
BF16 = mybir.dt.bfloat16
FP8 = mybir.dt.float8e4
AF = mybir.ActivationFunctionType
DR = mybir.MatmulPerfMode.DoubleRow

B, T, D = 4, 2048, 1024
H, HD = 16, 64
HG = 2                      # head groups (tensor-parallel factor)
H_LOC = H // HG             # 8 heads per core
DH = H_LOC * HD             # 512 local qkv width
N_CORES = 8
SCALE = 1.0 / np.sqrt(HD)
NEG_LN16 = -float(np.log(16.0))
FPAD = 80                   # fp8 V feature pad (65 -> 80, Ko stride % 16)


def build_attention(t_len=T, d_model=D, dh=DH):
    KC = d_model // 128          # contraction chunks for qkv
    NT = t_len // 128            # token tiles
    NQ = t_len // 512            # token chunks (= query chunks)
    NF = dh // 128               # feature tiles of q/k
    NH = dh // HD                # local heads
    KP = dh // 128               # contraction chunks for proj
    ND = d_model // 512          # output column chunks
    NPAIR = NT // 2              # key-tile pairs (fp8 DoubleRow)

    nc = bacc.Bacc("TRN2", target_bir_lowering=False, debug=False,
                   num_devices=N_CORES)

    x = nc.dram_tensor("x", [t_len, d_model], BF16, kind="ExternalInput")
    wq = nc.dram_tensor("wq", [d_model, dh], BF16, kind="ExternalInput")
    wk = nc.dram_tensor("wk", [d_model, dh], BF16, kind="ExternalInput")
    wv = nc.dram_tensor("wv", [d_model, dh], BF16, kind="ExternalInput")
    bqs = nc.dram_tensor("bqs", [dh], F32, kind="ExternalInput")  # pre-scaled
    bk = nc.dram_tensor("bk", [dh], F32, kind="ExternalInput")
    wp = nc.dram_tensor("wp", [dh, d_model], BF16, kind="ExternalInput")
    out = nc.dram_tensor("out", [t_len, d_model], F32, kind="ExternalOutput")

    with tile.TileContext(nc) as tc:
        with (
            tc.tile_pool(name="singles", bufs=1) as singles,
            tc.tile_pool(name="persist", bufs=1) as persist,
            tc.tile_pool(name="xt", bufs=2) as pool_xt,
            tc.tile_pool(name="st", bufs=4) as pool_st,
            tc.tile_pool(name="dn", bufs=3) as pool_dn,
            tc.tile_pool(name="bc", bufs=3) as pool_bc,
            tc.tile_pool(name="dnd", bufs=3, space="DRAM") as pool_dnd,
            tc.tile_pool(name="ostg", bufs=4) as pool_ostg,
            tc.tile_pool(name="ps_mm", bufs=2, space="PSUM") as ps_mm,
            tc.tile_pool(name="ps_st", bufs=2, space="PSUM") as ps_st,
            tc.tile_pool(name="ps_ot", bufs=2, space="PSUM") as ps_ot,
        ):
            ln16 = singles.tile([128, 1], F32)
            nc.vector.memset(ln16[:, :], NEG_LN16)

            bqs_sb = singles.tile([128, NF], F32)
            nc.sync.dma_start(bqs_sb, bqs.rearrange("(f p) -> p f", p=128))
            bk_sb = singles.tile([128, NF], F32)
            nc.sync.dma_start(bk_sb, bk.rearrange("(f p) -> p f", p=128))

            # resident weights; per-c-chunk DMAs so the first q/k matmuls
            # start as soon as their slice lands.
            wq_sb = singles.tile([128, KC, dh], BF16, tag="wq")
            wq_r = wq.rearrange("(c p) n -> p c n", p=128)
            wk_sb = singles.tile([128, KC, dh], BF16, tag="wk")
            wk_r = wk.rearrange("(c p) n -> p c n", p=128)
            for c in range(KC):
                nc.sync.dma_start(wq_sb[:, c, :], wq_r[:, c, :])
                nc.sync.dma_start(wk_sb[:, c, :], wk_r[:, c, :])
            wv_sb = singles.tile([128, KC, dh], BF16, tag="wv")
            wv_r = wv.rearrange("(c p) n -> p c n", p=128)
            for c in range(KC):
                nc.sync.dma_start(wv_sb[:, c, :], wv_r[:, c, :])
            wp_sb = singles.tile([128, KP, d_model], BF16, tag="wp")
            wp_r = wp.rearrange("(c p) n -> p c n", p=128)
            for c in range(KP):
                nc.sync.dma_start(wp_sb[:, c, :], wp_r[:, c, :])

            # persistent activations
            qT = persist.tile([128, NF, t_len], BF16, tag="qT")  # [feat, tok]
            kT = persist.tile([128, NF, t_len], BF16, tag="kT")
            # V in fp8, keyed [key, pair, u, head, feat]; feat 64 = ones
            # column (denominator), 65.. = zero pad.
            vaug = persist.tile([128, NPAIR, 2, NH, FPAD], FP8, tag="vaug")
            nc.vector.memset(vaug[:, :, :, :, HD:HD + 1], 1.0)
            nc.vector.memset(vaug[:, :, :, :, HD + 1:], 0.0)
            # bf16 V for the first 4 key tiles: query chunk 0 has short
            # attention rows (tiny n_eff) where fp8 weight/value
            # quantization is not averaged away, so chunk 0 runs in bf16.
            vb16 = persist.tile([128, 4, NH, HD + 2], BF16, tag="vb16")
            nc.vector.memset(vb16[:, :, :, HD:HD + 2], 1.0)
            oT = persist.tile([128, KP, t_len], BF16, tag="oT")

            def emit_proj(nq):
                for t in range(4 * nq, 4 * nq + 4):
                    for nn in range(ND):
                        pd = ps_mm.tile([128, 512], F32, tag="mm",
                                        name=f"pd{t}_{nn}")
                        for c in range(KP):
                            nc.tensor.matmul(
                                pd[:, :],
                                lhsT=oT[:, c, t * 128:(t + 1) * 128],
                                rhs=wp_sb[:, c, nn * 512:(nn + 1) * 512],
                                start=(c == 0), stop=(c == KP - 1))
                        ostg = pool_ostg.tile([128, 512], F32, tag="ostg",
                                              name=f"ostg{t}_{nn}")
                        nc.vector.tensor_copy(ostg[:, :], pd[:, :])
                        nc.sync.dma_start(
                            out[t * 128:(t + 1) * 128,
                                nn * 512:(nn + 1) * 512],
                            ostg[:, :])

            for n in range(NQ):
                # ---- transpose chunk n of x (DMA transpose, bf16) ----
                xt = pool_xt.tile([128, KC, 512], BF16, tag="xt",
                                  name=f"xt{n}")
                for dc in range(KC):
                    nc.sync.dma_start_transpose(
                        xt[:, dc, :],
                        x[n * 512:(n + 1) * 512, dc * 128:(dc + 1) * 128])

                # ---- q/k for chunk n ----
                for f in range(NF):
                    for which, w_sb, bias, dstT in (
                        ("q", wq_sb, bqs_sb, qT),
                        ("k", wk_sb, bk_sb, kT),
                    ):
                        pqk = ps_mm.tile([128, 512], F32, tag="mm",
                                         name=f"p_{which}{f}_{n}")
                        for c in range(KC):
                            nc.tensor.matmul(
                                pqk[:, :],
                                lhsT=w_sb[:, c, f * 128:(f + 1) * 128],
                                rhs=xt[:, c, :],
                                start=(c == 0), stop=(c == KC - 1))
                        nc.vector.tensor_scalar_add(
                            out=dstT[:, f, n * 512:(n + 1) * 512],
                            in0=pqk[:, :],
                            scalar1=bias[:, f:f + 1])

                # ---- V for chunk n (cast to fp8 in vaug) ----
                for tt in range(4):
                    t = 4 * n + tt
                    pv = ps_mm.tile([128, dh], F32, tag="mm", name=f"pv{t}")
                    for c in range(KC):
                        nc.tensor.matmul(
                            pv[:, :],
                            lhsT=xt[:, c, tt * 128:(tt + 1) * 128],
                            rhs=wv_sb[:, c, :],
                            start=(c == 0), stop=(c == KC - 1))
                    nc.vector.tensor_copy(
                        vaug[:, t // 2, t % 2, :, 0:HD],
                        pv.rearrange("p (h e) -> p h e", e=HD))
                    if t < 4:
                        nc.vector.tensor_copy(
                            vb16[:, t, :, 0:HD],
                            pv.rearrange("p (h e) -> p h e", e=HD))

                # ---- proj of the previous chunk ----
                if n > 0:
                    emit_proj(n - 1)

                # ---- attention for query chunk qj = n ----
                qj = n
                npair = 2 * qj + 2
                for h in range(NH):
                    f, rb = h // 2, (h % 2) * 64
                    pot = ps_ot.tile([FPAD, 512], F32, tag="ot",
                                     name=f"pot{h}_{qj}")
                    if qj == 0:
                        # bf16 path for the short-row chunk
                        for tp in range(npair):
                            pst = ps_st.tile([128, 2, 512], F32, tag="st",
                                             name=f"pst{h}_{qj}_{tp}")
                            stb = pool_st.tile([128, 2, 512], BF16,
                                               tag="stb",
                                               name=f"stb{h}_{tp}")
                            for u in range(2):
                                ti = 2 * tp + u
                                w = ti * 128
                                nc.tensor.matmul(
                                    pst[:, u, w:],
                                    lhsT=kT[rb:rb + 64, f,
                                            ti * 128:(ti + 1) * 128],
                                    rhs=qT[rb:rb + 64, f, w:512],
                                    start=True, stop=True)
                                nc.scalar.activation(
                                    stb[:, u, w:], pst[:, u, w:],
                                    AF.Exp, bias=ln16[:, 0:1])
                                nc.gpsimd.affine_select(
                                    out=stb[:, u, w:w + 128],
                                    in_=stb[:, u, w:w + 128],
                                    compare_op=mybir.AluOpType.is_ge,
                                    fill=0.0,
                                    base=0,
                                    channel_multiplier=-1,
                                    pattern=[[1, 128]])
                                nc.tensor.matmul(
                                    pot[0:HD + 1, w:],
                                    lhsT=vb16[:, ti, h, 0:HD + 1],
                                    rhs=stb[:, u, w:],
                                    start=(ti == 0), stop=(ti == 3))
                    else:
                        for tp in range(npair):
                            w0 = max(0, 256 * tp - 512 * qj)
                            diag = tp >= 2 * qj
                            pst = ps_st.tile([128, 2, 512], F32, tag="st",
                                             name=f"pst{h}_{qj}_{tp}")
                            st = pool_st.tile([128, 2, 512], FP8, tag="st",
                                              name=f"st{h}_{qj}_{tp}")
                            for u in range(2):
                                ti = 2 * tp + u
                                w = max(0, ti * 128 - qj * 512)
                                nc.tensor.matmul(
                                    pst[:, u, w:],
                                    lhsT=kT[rb:rb + 64, f,
                                            ti * 128:(ti + 1) * 128],
                                    rhs=qT[rb:rb + 64, f,
                                           qj * 512 + w:(qj + 1) * 512],
                                    start=True, stop=True)
                            if not diag:
                                nc.scalar.activation(st[:, :, :],
                                                     pst[:, :, :],
                                                     AF.Exp,
                                                     bias=ln16[:, 0:1])
                            else:
                                w1 = w0 + 128
                                nc.scalar.activation(st[:, 0, w0:],
                                                     pst[:, 0, w0:],
                                                     AF.Exp,
                                                     bias=ln16[:, 0:1])
                                nc.scalar.activation(st[:, 1, w1:],
                                                     pst[:, 1, w1:],
                                                     AF.Exp,
                                                     bias=ln16[:, 0:1])
                                nc.vector.memset(st[:, 1, w0:w1], 0.0)
                                for u in range(2):
                                    ti = 2 * tp + u
                                    w = ti * 128 - qj * 512
                                    nc.gpsimd.affine_select(
                                        out=st[:, u, w:w + 128],
                                        in_=st[:, u, w:w + 128],
                                        compare_op=mybir.AluOpType.is_ge,
                                        fill=0.0,
                                        base=0,
                                        channel_multiplier=-1,
                                        pattern=[[1, 128]])
                            nc.tensor.matmul(
                                pot[:, w0:],
                                lhsT=vaug[:, tp, :, h, :],
                                rhs=st[:, :, w0:],
                                start=(tp == 0), stop=(tp == npair - 1),
                                perf_mode=DR)
                    # normalize: reciprocal of the denominator row, DRAM
                    # bounce to broadcast it across 64 partitions, then a
                    # single fused multiply out of PSUM into bf16 oT.
                    dst = oT[rb:rb + 64, f, qj * 512:(qj + 1) * 512]
                    recip = pool_dn.tile([1, 512], F32, tag="dn",
                                         name=f"dn{h}_{qj}")
                    nc.vector.reciprocal(recip[:, :], pot[HD:HD + 1, :])
                    dnd = pool_dnd.tile([1, 512], F32, tag="dnd",
                                        name=f"dnd{h}_{qj}")
                    nc.sync.dma_start(dnd[:, :], recip[:, :])
                    bc = pool_bc.tile([64, 512], F32, tag="bc",
                                      name=f"bc{h}_{qj}")
                    flat = dnd.rearrange("p f -> (p f)")
                    bcast = bass.AP(tensor=flat.tensor, offset=flat.offset,
                                    ap=[[0, 64]] + list(flat.ap))
                    nc.sync.dma_start(bc[:, :], bcast)
                    nc.vector.tensor_mul(dst, pot[0:HD, :], bc[:, :])

            emit_proj(NQ - 1)

    nc.compile()
    return nc


_NC_CACHE = {}


def _get_nc():
    if "nc" not in _NC_CACHE:
        _NC_CACHE["nc"] = build_attention()
    return _NC_CACHE["nc"]


def shard_inputs(x, W_qkv, b_qkv, W_proj):
    bf = ml_dtypes.bfloat16
    in_maps = []
    for c in range(N_CORES):
        b, hg = divmod(c, HG)
        cs = slice(hg * DH, (hg + 1) * DH)
        m = {
            "x": np.ascontiguousarray(x[b]).astype(bf),
            "wq": (np.ascontiguousarray(W_qkv[:, 0 * D:1 * D][:, cs])
                   * np.float32(SCALE)).astype(bf),
            "wk": np.ascontiguousarray(W_qkv[:, 1 * D:2 * D][:, cs]).astype(bf),
            "wv": np.ascontiguousarray(W_qkv[:, 2 * D:3 * D][:, cs]).astype(bf),
            "bqs": np.ascontiguousarray(b_qkv[0 * D:1 * D][cs]) * np.float32(SCALE),
            "bk": np.ascontiguousarray(b_qkv[1 * D:2 * D][cs]),
            "wp": np.ascontiguousarray(W_proj[cs, :]).astype(bf),
        }
        in_maps.append(m)
    return in_maps


def kernel(x, W_qkv, b_qkv, W_proj, b_proj, _trace=False, _trace_kwargs=None):
    x = np.asarray(x, dtype=np.float32)
    W_qkv = np.asarray(W_qkv, dtype=np.float32)
    b_qkv = np.asarray(b_qkv, dtype=np.float32)
    W_proj = np.asarray(W_proj, dtype=np.float32)
    b_proj = np.asarray(b_proj, dtype=np.float32)

    nc = _get_nc()
    in_maps = shard_inputs(x, W_qkv, b_qkv, W_proj)
    res = bass_utils.run_bass_kernel_spmd(
        nc, in_maps, core_ids=list(range(N_CORES)),
        trace=_trace, **(_trace_kwargs or {}))

    # v-bias folded through the projection (b_v @ W_proj), plus b_proj.
    bias_full = b_qkv[2 * D:3 * D] @ W_proj + b_proj
    out = np.empty((B, T, D), dtype=np.float32)
    for b in range(B):
        acc = res.results[HG * b]["out"].astype(np.float32)
        for hg in range(1, HG):
            acc = acc + res.results[HG * b + hg]["out"]
        out[b] = acc + bias_full[None, :]
    if _trace:
        return out, res
    return out
